# revision 19
# baseline (speedup 1.0000x reference)
"""Tensor-parallel MultiHeadAttention (LN + fused QKV + causal SDPA + proj)
for 8 Trainium2 NeuronCores.

Sharding: 2 heads per core. LayerNorm gamma/beta folded into qkv weights on
host; LN (x-mu)*rstd applied via rank-1 PSUM corrections + evacuation scaling.
All heavy matmuls run in fp32r (1 cyc/row). Causal softmax computed on
transposed scores (scoresT[t,s]) so the softmax reduction is a PE ones-matmul.
Output projection partial sums are ReduceScattered across cores; host
reassembles the full [S,1,HID] output.
"""

import sys

sys.path.insert(0, "/opt/trn_rl_repo")

import math

import numpy as np

S, HID, NH, HD = 2048, 2048, 16, 128
EPS = 1e-5
NCORES = 8
HPC = NH // NCORES        # heads per core: 2
OQK = 2 * HPC * HD        # q+k rows per core: 512
OV = HPC * HD             # v rows per core: 256
KO = HID // 128           # contraction chunks: 16
NSB = S // 512            # s-blocks: 4
NTB = S // 128            # t-blocks: 16
RS_OUT = 512 // NCORES    # rows per core per RS chunk: 64
SCALE = 1.0 / math.sqrt(HD)
MASKVAL = -30000.0

_CACHE = {}


def _build_nc(debug=False):
    import concourse.mybir as mybir
    import concourse.tile as tile
    from concourse import bacc
    from contextlib import ExitStack

    f32 = mybir.dt.float32
    f32r = mybir.dt.float32r
    bf16 = mybir.dt.bfloat16
    Act = mybir.ActivationFunctionType

    nc = bacc.Bacc(num_devices=NCORES)

    # ---- I/O ----
    xT_d = nc.dram_tensor("xT", [HID, S], f32r, kind="ExternalInput")
    wqkT_d = nc.dram_tensor("wqkT", [HID, OQK], f32r, kind="ExternalInput")
    wvT_d = nc.dram_tensor("wvT", [HID, OV], f32r, kind="ExternalInput")
    wpT_d = nc.dram_tensor("wpT", [OV, HID], f32r, kind="ExternalInput")
    rsum_qk_d = nc.dram_tensor("rsum_qk", [1, OQK], f32r, kind="ExternalInput")
    rsum_v_d = nc.dram_tensor("rsum_v", [1, OV], f32r, kind="ExternalInput")
    bqk_d = nc.dram_tensor("bqk", [1, OQK], f32r, kind="ExternalInput")
    bv_d = nc.dram_tensor("bv", [1, OV], f32r, kind="ExternalInput")
    pbias8_d = nc.dram_tensor("pbias8", [1, HID], f32, kind="ExternalInput")
    maskneg_d = nc.dram_tensor("maskneg", [128, 128], bf16, kind="ExternalInput")
    ident_d = nc.dram_tensor("ident", [128, 128], bf16, kind="ExternalInput")
    ones_d = nc.dram_tensor("ones_col", [128, 1], f32r, kind="ExternalInput")
    out_d = nc.dram_tensor("out", [NSB * RS_OUT, HID], f32, kind="ExternalOutput")

    dbg = {}
    if debug:
        dbg["qkT"] = nc.dram_tensor("dbg_qkT", [OQK, S], f32, kind="ExternalOutput")
        dbg["v"] = nc.dram_tensor("dbg_v", [S, OV], f32, kind="ExternalOutput")
        dbg["stats"] = nc.dram_tensor("dbg_stats", [4, S], f32, kind="ExternalOutput")
        dbg["ctxT"] = nc.dram_tensor("dbg_ctxT", [HPC * HD, S], f32, kind="ExternalOutput")
        dbg["partial"] = nc.dram_tensor("dbg_partial", [S, HID], f32, kind="ExternalOutput")

    # internal DRAM for collectives + stats round trips
    rstd_dram = nc.dram_tensor("rstd_scratch", [NSB, 512], f32)
    rden_dram = nc.dram_tensor("rden_scratch", [HPC, NSB, 512], f32)
    part_dram = [nc.dram_tensor(f"part{sb}", [512, HID], f32) for sb in range(NSB)]
    rs_dram = [
        nc.dram_tensor(f"rsout{sb}", [RS_OUT, HID], f32) for sb in range(NSB)
    ]

    ctx = ExitStack()
    with ctx:
        tc = ctx.enter_context(tile.TileContext(nc))
        # resident pools (whole kernel lifetime)
        wpool = ctx.enter_context(tc.tile_pool(name="wpool", bufs=1))
        rows = ctx.enter_context(tc.tile_pool(name="rows", bufs=1))
        bigout = ctx.enter_context(tc.tile_pool(name="bigout", bufs=1))
        statrow = ctx.enter_context(tc.tile_pool(name="statrow", bufs=1))

        # ---- resident loads ----
        wpT = wpool.tile([128, HPC, HID], f32r)
        nc.sync.dma_start(out=wpT, in_=wpT_d.rearrange("(c p) o -> p c o", p=128))
        rsum_qk = rows.tile([1, OQK], f32r)
        nc.sync.dma_start(out=rsum_qk, in_=rsum_qk_d[:, :])
        rsum_v = rows.tile([1, OV], f32r)
        nc.sync.dma_start(out=rsum_v, in_=rsum_v_d[:, :])
        bqk = rows.tile([1, OQK], f32r)
        nc.sync.dma_start(out=bqk, in_=bqk_d[:, :])
        bv = rows.tile([1, OV], f32r)
        nc.sync.dma_start(out=bv, in_=bv_d[:, :])
        maskneg = rows.tile([128, 128], bf16)
        nc.sync.dma_start(out=maskneg, in_=maskneg_d[:, :])
        ident = rows.tile([128, 128], bf16)
        nc.sync.dma_start(out=ident, in_=ident_d[:, :])
        ones_col = rows.tile([128, 1], f32r)
        nc.sync.dma_start(out=ones_col, in_=ones_d[:, :])
        pbias8_b = rows.tile([128, HID], f32)
        nc.sync.dma_start(out=pbias8_b, in_=pbias8_d[:, :].to_broadcast([128, HID]))
        eps_tile = rows.tile([128, 1], f32)
        nc.vector.memset(eps_tile, EPS)

        # ---- persistent phase-1 outputs ----
        qkT = [bigout.tile([128, S], f32r, name=f"qkT{ob}") for ob in range(4)]
        vtile = bigout.tile([128, NTB, OV], f32r, name="vtile")
        ctxT = [bigout.tile([128, S], f32r, name=f"ctxT{h}") for h in range(HPC)]
        rstd_col = bigout.tile([128, NSB * 4], f32, name="rstd_col")
        if debug:
            stats_dbg = bigout.tile([4, S], f32, name="stats_dbg")

        # =========================================================
        # Phase 1: stats + qkT + v  (per s-block of 512)
        # =========================================================
        with (
            tc.tile_pool(name="wqkv", bufs=1) as wqkv,
            tc.tile_pool(name="xpool", bufs=4) as xpool,
            tc.tile_pool(name="sqpool", bufs=3) as sqpool,
            tc.tile_pool(name="rowr", bufs=2) as rowr,
            tc.tile_pool(name="bc1", bufs=2) as bc1,
            tc.tile_pool(name="ps_qk", bufs=4, space="PSUM") as ps_qk_pool,
            tc.tile_pool(name="ps_v", bufs=2, space="PSUM") as ps_v_pool,
            tc.tile_pool(name="ps_st", bufs=2, space="PSUM") as ps_st_pool,
        ):
            wqkT = wqkv.tile([128, KO, OQK], f32r)
            nc.sync.dma_start(
                out=wqkT, in_=wqkT_d.rearrange("(ko p) o -> p ko o", p=128)
            )
            wvT = wqkv.tile([128, KO, OV], f32r)
            nc.sync.dma_start(
                out=wvT, in_=wvT_d.rearrange("(ko p) o -> p ko o", p=128)
            )

            for sb in range(NSB):
                s0 = sb * 512
                ps_sums = ps_st_pool.tile([1, 512], f32, tag="ps_st", name="ps_sums")
                ps_sumsq = ps_st_pool.tile([1, 512], f32, tag="ps_st", name="ps_sumsq")
                ps_qk = [
                    ps_qk_pool.tile([128, 512], f32, tag="ps_qk", name=f"ps_qk{ob}")
                    for ob in range(4)
                ]
                # two [128,512] banks, each packing two 256-wide v groups
                ps_v = [
                    ps_v_pool.tile([128, 512], f32, tag="ps_v", name=f"ps_v{i}")
                    for i in range(2)
                ]
                for h in range(KO):
                    xt = xpool.tile([128, 512], f32r, tag="xt", name=f"xt{sb}_{h}")
                    nc.sync.dma_start(
                        out=xt, in_=xT_d[h * 128 : (h + 1) * 128, s0 : s0 + 512]
                    )
                    xsq = sqpool.tile([128, 512], f32r, tag="xsq")
                    if h % 2 == 0:
                        nc.scalar.activation(out=xsq, in_=xt, func=Act.Square)
                    else:
                        nc.vector.tensor_mul(out=xsq, in0=xt, in1=xt)
                    nc.tensor.matmul(
                        ps_sums, ones_col, xt, start=(h == 0), stop=(h == KO - 1)
                    )
                    nc.tensor.matmul(
                        ps_sumsq, ones_col, xsq, start=(h == 0), stop=(h == KO - 1)
                    )
                    for ob in range(4):
                        nc.tensor.matmul(
                            ps_qk[ob],
                            wqkT[:, h, ob * 128 : (ob + 1) * 128],
                            xt,
                            start=(h == 0),
                            stop=False,
                        )
                    for vs in range(4):
                        # odd vs rides the even group's h==0 bank clear
                        nc.tensor.matmul(
                            ps_v[vs // 2][:, (vs % 2) * 256 : (vs % 2 + 1) * 256],
                            xt[:, vs * 128 : (vs + 1) * 128],
                            wvT[:, h, :],
                            start=(h == 0 and vs % 2 == 0),
                            stop=False,
                            skip_group_check=(vs % 2 == 1),
                        )

                # stats rows
                mu = statrow.tile([1, 512], f32, tag="mu")
                nc.vector.tensor_scalar_mul(out=mu, in0=ps_sums, scalar1=1.0 / HID)
                msq = statrow.tile([1, 512], f32, tag="msq")
                nc.vector.tensor_scalar_mul(out=msq, in0=ps_sumsq, scalar1=1.0 / HID)
                mu2 = statrow.tile([1, 512], f32, tag="mu2")
                nc.vector.tensor_mul(out=mu2, in0=mu, in1=mu)
                var = statrow.tile([1, 512], f32, tag="var")
                nc.vector.tensor_sub(out=var, in0=msq, in1=mu2)
                invrstd = statrow.tile([1, 512], f32, tag="invrstd")
                nc.scalar.activation(
                    out=invrstd, in_=var, func=Act.Sqrt, bias=eps_tile[0:1]
                )
                rstd = statrow.tile([1, 512], f32, tag="rstd")
                nc.vector.reciprocal(out=rstd, in_=invrstd)
                negmu_r = rowr.tile([1, 512], f32r, tag="negmu_r")
                nc.vector.tensor_scalar_mul(out=negmu_r, in0=mu, scalar1=-1.0)
                invrstd_r = rowr.tile([1, 512], f32r, tag="invrstd_r")
                nc.vector.tensor_copy(out=invrstd_r, in_=invrstd)

                if debug:
                    nc.vector.tensor_copy(out=stats_dbg[0:1, s0 : s0 + 512], in_=mu)
                    nc.vector.tensor_copy(out=stats_dbg[1:2, s0 : s0 + 512], in_=var)
                    nc.vector.tensor_copy(out=stats_dbg[2:3, s0 : s0 + 512], in_=rstd)
                    nc.vector.tensor_copy(out=stats_dbg[3:4, s0 : s0 + 512], in_=invrstd)

                # rstd -> DRAM -> column layout + broadcast rows
                nc.sync.dma_start(out=rstd_dram[sb : sb + 1, :], in_=rstd)
                nc.sync.dma_start(
                    out=rstd_col[:, sb * 4 : (sb + 1) * 4],
                    in_=rstd_dram[sb, :].rearrange("(f p) -> p f", p=128),
                )
                rstd_b = bc1.tile([128, 512], f32, tag="rstd_b")
                nc.sync.dma_start(
                    out=rstd_b,
                    in_=rstd_dram[sb : sb + 1, :].to_broadcast([128, 512]),
                )

                # qk rank-1 corrections + evac
                for ob in range(4):
                    nc.tensor.matmul(
                        ps_qk[ob],
                        rsum_qk[0:1, ob * 128 : (ob + 1) * 128],
                        negmu_r,
                        start=False,
                        stop=False,
                    )
                    nc.tensor.matmul(
                        ps_qk[ob],
                        bqk[0:1, ob * 128 : (ob + 1) * 128],
                        invrstd_r,
                        start=False,
                        stop=True,
                    )
                    nc.vector.tensor_mul(
                        out=qkT[ob][:, s0 : s0 + 512], in0=ps_qk[ob], in1=rstd_b
                    )

                # v rank-1 corrections + evac
                for vs in range(4):
                    pv = ps_v[vs // 2][:, (vs % 2) * 256 : (vs % 2 + 1) * 256]
                    nc.tensor.matmul(
                        pv,
                        negmu_r[0:1, vs * 128 : (vs + 1) * 128],
                        rsum_v,
                        start=False,
                        stop=False,
                        skip_group_check=True,
                    )
                    nc.tensor.matmul(
                        pv,
                        invrstd_r[0:1, vs * 128 : (vs + 1) * 128],
                        bv,
                        start=False,
                        stop=True,
                        skip_group_check=True,
                    )
                    nc.vector.tensor_scalar_mul(
                        out=vtile[:, sb * 4 + vs, :],
                        in0=pv,
                        scalar1=rstd_col[:, sb * 4 + vs : sb * 4 + vs + 1],
                    )

        # =========================================================
        # Phase 2+3: attention per (sb, head), then proj + RS per sb
        # =========================================================
        with (
            tc.tile_pool(name="exppool", bufs=4) as exppool,
            tc.tile_pool(name="bc2", bufs=2) as bc2,
            tc.tile_pool(name="projpool", bufs=3) as projpool,
            tc.tile_pool(name="rstpool", bufs=2) as rstpool,
            tc.tile_pool(name="dbgpool", bufs=1) as dbgpool,
            tc.tile_pool(name="ps_sc", bufs=2, space="PSUM") as ps_sc_pool,
            tc.tile_pool(name="ps_ctx", bufs=2, space="PSUM") as ps_ctx_pool,
            tc.tile_pool(name="ps_den", bufs=2, space="PSUM") as ps_den_pool,
            tc.tile_pool(name="ps_pr", bufs=2, space="PSUM") as ps_pr_pool,
        ):
            if debug:
                for ob in range(4):
                    qf = dbgpool.tile([128, S], f32, tag="dbgq", bufs=2)
                    nc.vector.tensor_copy(out=qf, in_=qkT[ob])
                    nc.sync.dma_start(
                        out=dbg["qkT"][ob * 128 : (ob + 1) * 128, :], in_=qf
                    )
                vf = dbgpool.tile([128, NTB, OV], f32, tag="dbgv")
                nc.vector.tensor_copy(out=vf, in_=vtile)
                nc.sync.dma_start(
                    out=dbg["v"].rearrange("(tb p) o -> p tb o", p=128), in_=vf
                )
                nc.sync.dma_start(out=dbg["stats"][:, :], in_=stats_dbg)

            for sb in range(NSB):
                s0 = sb * 512
                ntb = 4 * (sb + 1)  # causal t-blocks
                for h in range(HPC):
                    qT = qkT[h]
                    kT = qkT[2 + h]
                    ps_ctx = ps_ctx_pool.tile([128, 512], f32, tag="ps_ctx", name=f"ps_ctx{sb}_{h}")
                    ps_den = ps_den_pool.tile([1, 512], f32, tag="ps_den", name=f"ps_den{sb}_{h}")
                    for tb in range(ntb):
                        t0 = tb * 128
                        delta = max(0, t0 - s0)
                        ps_sc = ps_sc_pool.tile([128, 512], f32, tag="ps_sc", name="ps_sc")
                        nc.tensor.matmul(
                            ps_sc[:, delta:512],
                            kT[:, t0 : t0 + 128],
                            qT[:, s0 + delta : s0 + 512],
                            start=True,
                            stop=(t0 < s0),
                        )
                        if t0 >= s0:
                            nc.tensor.matmul(
                                ps_sc[:, delta : delta + 128],
                                maskneg,
                                ident,
                                start=False,
                                stop=True,
                            )
                        expt = exppool.tile([128, 512], f32r, tag="expt")
                        nc.scalar.activation(
                            out=expt[:, delta:512],
                            in_=ps_sc[:, delta:512],
                            func=Act.Exp,
                            scale=SCALE,
                        )
                        # columns [0, delta) are invalid (t > s) and simply
                        # never written: every column's first accumulant is
                        # tb == 0 (delta == 0), so start=True covers all.
                        nc.tensor.matmul(
                            ps_ctx[:, delta:512],
                            vtile[:, tb, h * HD : (h + 1) * HD],
                            expt[:, delta:512],
                            start=(tb == 0),
                            stop=(tb == ntb - 1),
                            skip_group_check=True,
                        )
                        nc.tensor.matmul(
                            ps_den[:, delta:512],
                            ones_col,
                            expt[:, delta:512],
                            start=(tb == 0),
                            stop=(tb == ntb - 1),
                            skip_group_check=True,
                        )
                    rden = statrow.tile([1, 512], f32, tag="rden")
                    nc.vector.reciprocal(out=rden, in_=ps_den)
                    nc.sync.dma_start(out=rden_dram[h, sb : sb + 1, :], in_=rden)
                    rden_b = bc2.tile([128, 512], f32, tag="rden_b")
                    nc.sync.dma_start(
                        out=rden_b,
                        in_=rden_dram[h, sb : sb + 1, :].to_broadcast([128, 512]),
                    )
                    nc.vector.tensor_mul(
                        out=ctxT[h][:, s0 : s0 + 512], in0=ps_ctx, in1=rden_b
                    )

                # proj for this sb
                for st_i in range(4):
                    sg = s0 + st_i * 128
                    for ob in range(4):
                        o0 = ob * 512
                        ps_pr = ps_pr_pool.tile([128, 512], f32, tag="ps_pr", name="ps_pr")
                        for h in range(HPC):
                            nc.tensor.matmul(
                                ps_pr,
                                ctxT[h][:, sg : sg + 128],
                                wpT[:, h, o0 : o0 + 512],
                                start=(h == 0),
                                stop=(h == HPC - 1),
                            )
                        ptile = projpool.tile([128, 512], f32, tag="ptile")
                        nc.vector.tensor_add(
                            out=ptile, in0=ps_pr, in1=pbias8_b[:, o0 : o0 + 512]
                        )
                        nc.sync.dma_start(
                            out=part_dram[sb][
                                st_i * 128 : (st_i + 1) * 128, o0 : o0 + 512
                            ],
                            in_=ptile,
                        )
                        if debug:
                            nc.sync.dma_start(
                                out=dbg["partial"][sg : sg + 128, o0 : o0 + 512],
                                in_=ptile,
                            )

                nc.gpsimd.collective_compute(
                    "ReduceScatter",
                    mybir.AluOpType.add,
                    replica_groups=[list(range(NCORES))],
                    ins=[part_dram[sb].ap()],
                    outs=[rs_dram[sb].ap()],
                )
                rst = rstpool.tile([128, RS_OUT * HID // 128], f32, tag="rst")
                nc.sync.dma_start(
                    out=rst,
                    in_=rs_dram[sb].rearrange("a (two b) -> (a two) b", two=2),
                )
                nc.sync.dma_start(
                    out=out_d[sb * RS_OUT : (sb + 1) * RS_OUT, :].rearrange(
                        "a (two b) -> (a two) b", two=2
                    ),
                    in_=rst,
                )

            if debug:
                for h in range(HPC):
                    cf = dbgpool.tile([128, S], f32, tag="dbgq", bufs=2)
                    nc.vector.tensor_copy(out=cf, in_=ctxT[h])
                    nc.sync.dma_start(
                        out=dbg["ctxT"][h * 128 : (h + 1) * 128, :], in_=cf
                    )

    nc.finalize()
    return nc


def get_nc(debug=False):
    key = ("nc", debug)
    if key not in _CACHE:
        _CACHE[key] = _build_nc(debug=debug)
    return _CACHE[key]


def make_in_maps(hidden_states, ln_weight, ln_bias, qkv_weight, qkv_bias,
                 proj_weight, proj_bias):
    import ml_dtypes

    f4 = np.float32
    x = np.asarray(hidden_states, f4)[:, 0, :]                      # [S, HID]
    xT = np.ascontiguousarray(x.T)                                  # [HID, S]
    g = np.asarray(ln_weight, f4)
    b = np.asarray(ln_bias, f4)
    W = np.asarray(qkv_weight, f4)
    W1 = W * g[None, :]
    b1 = np.asarray(qkv_bias, f4) + W @ b
    W3 = W1.reshape(3, NH, HD, HID)
    b3 = b1.reshape(3, NH, HD)
    pw = np.asarray(proj_weight, f4)
    pb8 = (np.asarray(proj_bias, f4) / NCORES).reshape(1, HID)
    maskneg = np.triu(np.full((128, 128), MASKVAL, f4), 1).astype(ml_dtypes.bfloat16)
    ident = np.eye(128, dtype=ml_dtypes.bfloat16)
    ones_col = np.ones((128, 1), f4)

    in_maps = []
    for c in range(NCORES):
        hs = slice(HPC * c, HPC * (c + 1))
        Wq = W3[0, hs].reshape(OV, HID)
        Wk = W3[1, hs].reshape(OV, HID)
        Wv = W3[2, hs].reshape(OV, HID)
        Wqk = np.concatenate([Wq, Wk], 0)                           # [512, HID]
        in_maps.append({
            "xT": xT,
            "wqkT": np.ascontiguousarray(Wqk.T),
            "wvT": np.ascontiguousarray(Wv.T),
            "wpT": np.ascontiguousarray(pw[:, OV * c : OV * (c + 1)].T),
            "rsum_qk": Wqk.sum(1).reshape(1, OQK),
            "rsum_v": Wv.sum(1).reshape(1, OV),
            "bqk": np.concatenate(
                [b3[0, hs].reshape(OV), b3[1, hs].reshape(OV)]
            ).reshape(1, OQK),
            "bv": b3[2, hs].reshape(1, OV),
            "pbias8": pb8,
            "maskneg": maskneg,
            "ident": ident,
            "ones_col": ones_col,
        })
    return in_maps


def assemble(outs):
    """outs: list of per-core [NSB*RS_OUT, HID] arrays -> full [S, 1, HID]."""
    full = np.empty((S, HID), np.float32)
    for c in range(NCORES):
        o = outs[c]
        for sb in range(NSB):
            full[sb * 512 + c * RS_OUT : sb * 512 + (c + 1) * RS_OUT, :] = o[
                sb * RS_OUT : (sb + 1) * RS_OUT, :
            ]
    return full.reshape(S, 1, HID)


def kernel(hidden_states, ln_weight, ln_bias, qkv_weight, qkv_bias,
           proj_weight, proj_bias):
    from concourse.bass_utils import run_bass_kernel_spmd

    in_maps = make_in_maps(hidden_states, ln_weight, ln_bias, qkv_weight,
                           qkv_bias, proj_weight, proj_bias)
    nc = get_nc()
    res = run_bass_kernel_spmd(nc, in_maps, core_ids=list(range(NCORES)))
    return assemble([res.results[c]["out"] for c in range(NCORES)])
